# revision 21
# baseline (speedup 1.0000x reference)
"""Pooled-KV attention block on 8 Trainium2 cores, data-parallel over batch.

Reference computation (per batch element b, with x_b: [64, 64, 512] -> [4096, 512]):
    f  = x_b @ wf                     # [4096, 64]
    xp = avgpool2x2(x_b)              # [1024, 512]
    g  = xp @ wg                      # [1024, 64]
    h  = xp @ wh                      # [1024, 256]
    a  = softmax(f @ g.T, axis=-1)    # [4096, 1024]
    y  = a @ h                        # [4096, 256]
    out = y @ wo                      # [4096, 512]

Kernel strategy (one core per batch element, weights replicated). The PE
is the bottleneck (~176k columns of fp16 matmul at 1 col/cycle), so
everything else is arranged to keep it streaming back-to-back:
  - Host supplies x transposed as fp16 (xT [512, 4096]), the 2x2 average
    pooled map xpT [512, 1024] (pooling is linear - fold it into the
    host), and weights pre-transposed to [partition, kc, free] so every
    DMA is a contiguous >=1KB-per-partition burst (strided weight
    rearranges cost ~4us of queue issue time each).
  - A ~7us burst of throwaway matmuls runs while the first input DMAs
    land: it ramps the tensor engine's DVFS clock (otherwise the first
    tiles run at the mid p-state, ~20% slower) and pre-loads the Exp
    activation table.
  - All compute matmuls fp16; intermediates transposed: fT/gT carry
    duplicated 64-row groups so score matmuls pack two 128-key chunks
    into disjoint PE row groups; h is [m, 256] with keys on partitions;
    y accumulates as [e, n] in two psum banks.
  - Softmax skips max-subtraction (|scores| < ~6, exp(fp16) safe). Row
    sums ride a DoubleRow fp8 ones-matmul (2x the fp16 rate): each fp16
    exp tile is copied to e4m3 by a gpsimd DMA (DMA-side dtype cast -
    free engine time), two pairs behind the y-matmuls to hide the DMA
    latency. Quantizing only the row sums costs ~3e-3 relative error.
  - 1/rowsum is transposed to per-partition layout via a DRAM bounce
    (hidden one tile later) and folded into the output-projection drain
    as a per-partition scalar multiply (DVE/Act alternate), which also
    casts to fp16 for the store. The final tile instead transposes its
    row-sum vector on the idle PE (transpose-mode matmuls) to keep the
    ~4us bounce off the tail.
  - Output stored fp16 (host upcasts), halving write traffic; stores
    coalesced per 512-row tile, final tile chunked per 128 rows.
"""

import sys
import types

import numpy as np

import concourse.mybir as mybir
import concourse.tile as tile
from concourse import bacc
from concourse.bass_utils import run_bass_kernel_spmd

# If BASS_TRACE is set but this image's antenv lacks axon_hooks, bass_utils
# would crash on import; provide a no-op hook module so tracing degrades
# gracefully instead (a real hook installed earlier, e.g. by test.py, wins).
try:
    import antenv.axon_hooks  # noqa: F401
except ImportError:
    import antenv

    _stub = types.ModuleType("antenv.axon_hooks")
    _stub._hook = None
    _stub.set_axon_ntff_profile_hook = lambda h: setattr(_stub, "_hook", h)
    _stub.get_axon_ntff_profile_hook = lambda: _stub._hook
    sys.modules["antenv.axon_hooks"] = _stub
    antenv.axon_hooks = _stub

F32 = mybir.dt.float32
F16 = mybir.dt.float16
F8 = mybir.dt.float8e4

P = 128          # SBUF partitions
C = 512          # channels
KC = C // P      # 4 contraction chunks over channels
N = 4096         # query positions (64*64)
NTILE = 512      # n tile (psum free dim)
NT = N // NTILE  # 8 n tiles
M = 1024         # pooled key positions (32*32)
MC = M // P      # 8 key chunks
NP = MC // 2     # 4 score-chunk pairs per n tile
D = 64           # qk head dim
E = 256          # value dim (C//2)
EC = E // P      # 2 value chunks

_CACHE = {}
_EYE = np.eye(128, dtype=np.float32)


def _build():
    nc = bacc.Bacc(None, target_bir_lowering=False)

    xt_d = nc.dram_tensor("xt", [C, N], F16, kind="ExternalInput")
    xp_d = nc.dram_tensor("xp", [C, M], F16, kind="ExternalInput")
    # weights pre-transposed on host to [partition, kc, free] contiguous
    wf_d = nc.dram_tensor("wfr", [P, KC * P], F16, kind="ExternalInput")
    wg_d = nc.dram_tensor("wgr", [P, KC * P], F16, kind="ExternalInput")
    wh_d = nc.dram_tensor("whr", [P, KC * E], F16, kind="ExternalInput")
    wo_d = nc.dram_tensor("wor", [P, EC * C], F16, kind="ExternalInput")
    eye_d = nc.dram_tensor("eye", [P, P], F32, kind="ExternalInput")
    out_d = nc.dram_tensor("out", [N, C], F16, kind="ExternalOutput")

    with tile.TileContext(nc) as tc:
        with (
            tc.tile_pool(name="const", bufs=1) as const_pool,
            tc.tile_pool(name="exp", bufs=4) as exp_pool,
            tc.tile_pool(name="exp8", bufs=4) as exp8_pool,
            tc.tile_pool(name="ysb", bufs=2) as y_pool,
            tc.tile_pool(name="osb", bufs=2) as o_pool,
            tc.tile_pool(name="small", bufs=2) as small_pool,
            tc.tile_pool(name="ps_pair", bufs=2, space="PSUM") as ps_pair_pool,
            tc.tile_pool(name="ps_y0", bufs=1, space="PSUM") as ps_y0_pool,
            tc.tile_pool(name="ps_y1", bufs=1, space="PSUM") as ps_y1_pool,
            tc.tile_pool(name="ps_sum", bufs=1, space="PSUM") as ps_sum_pool,
            tc.tile_pool(name="ps_work", bufs=1, space="PSUM") as ps_work_pool,
            tc.tile_pool(name="scr", bufs=1, space="DRAM") as scr_pool,
        ):
            xt_sb = const_pool.tile([P, KC, N], F16)
            xp_sb = const_pool.tile([P, KC, M], F16)
            wf_sb = const_pool.tile([P, KC, P], F16)
            wg_sb = const_pool.tile([P, KC, P], F16)
            wh_sb = const_pool.tile([P, KC, E], F16)
            wo_sb = const_pool.tile([P, EC, C], F16)
            ones_sb = const_pool.tile([P, 2, P], F8)
            eye_sb = const_pool.tile([P, P], F32)
            fT_sb = const_pool.tile([P, N], F16)
            gT_sb = const_pool.tile([P, M], F16)
            h_sb = const_pool.tile([P, MC, E], F16)

            scr = scr_pool.tile([NT, NTILE], F32)  # reciprocal transpose bounce

            nc.vector.memset(ones_sb, 1.0)  # fp8 ones for DoubleRow sums

            # ---- input DMA, spread over 4 HWDGE queues ----
            # weights first (tiny), then xp (gates g/h -> whole attention),
            # then xt n-tiles in consumption order.
            # xt loads at quarter granularity with kc-paired DMAs so each
            # partition row is a contiguous 2 KB burst (full ring rate);
            # weights + first quarter + xp spread across all three queues
            # so the attention pipeline unblocks ~15 us in.
            xt_r = xt_d.rearrange("(kc p) n -> p kc n", p=P)
            xp_r = xp_d.rearrange("(kc p) m -> p kc m", p=P)
            NQ = N // 4

            def load_quarter_a(eng, q):
                qsl = slice(q * NQ, (q + 1) * NQ)
                eng.dma_start(xt_sb[:, 0:2, qsl], xt_r[:, 0:2, qsl])

            def load_quarter_b(eng, q):
                qsl = slice(q * NQ, (q + 1) * NQ)
                eng.dma_start(xt_sb[:, 2:4, qsl], xt_r[:, 2:4, qsl])

            load_quarter_a(nc.sync, 0)
            load_quarter_b(nc.scalar, 0)
            nc.gpsimd.dma_start(xp_sb[:, 0:2, :], xp_r[:, 0:2, :])
            nc.gpsimd.dma_start(xp_sb[:, 2:4, :], xp_r[:, 2:4, :])
            nc.scalar.dma_start(
                wf_sb, wf_d.rearrange("p (kc d) -> p kc d", kc=KC)
            )
            nc.scalar.dma_start(
                wg_sb, wg_d.rearrange("p (kc d) -> p kc d", kc=KC)
            )
            nc.scalar.dma_start(
                wh_sb, wh_d.rearrange("p (kc e) -> p kc e", kc=KC)
            )
            nc.scalar.dma_start(
                wo_sb, wo_d.rearrange("p (ec c) -> p ec c", ec=EC)
            )
            load_quarter_a(nc.sync, 1)
            load_quarter_b(nc.scalar, 1)
            load_quarter_a(nc.sync, 2)
            load_quarter_b(nc.sync, 2)
            load_quarter_a(nc.sync, 3)
            load_quarter_b(nc.sync, 3)
            nc.sync.dma_start(eye_sb, eye_d[:, :])

            # PE warm-up: ~5 us of throwaway matmuls while the first input
            # DMAs land, so the tensor-engine clock is ramped before real
            # work; also loads the Exp activation table off the critical path
            warm_sb = const_pool.tile([P, NTILE], F16)
            nc.vector.memset(warm_sb, 0.0)
            ps_warm = ps_work_pool.tile([P, C], F32, tag="ps_work",
                                        name="ps_warm")
            NWARM = 36
            for i in range(NWARM):
                nc.tensor.matmul(
                    ps_warm, lhsT=warm_sb[:, 0:P], rhs=warm_sb,
                    start=(i == 0), stop=(i == NWARM - 1),
                )
            warm_e = small_pool.tile([1, 2], F32, tag="warm_e")
            nc.scalar.activation(warm_e, ps_warm[0:1, 0:2],
                                 mybir.ActivationFunctionType.Exp)

            # ---- projection helpers (PE + DVE drain) ----
            def f_tile(nt):
                ps = ps_pair_pool.tile([P, 2 * NTILE], F32, tag="ps_pair",
                                       name=f"ps_f{nt}")
                nsl = slice(nt * NTILE, (nt + 1) * NTILE)
                for kc in range(KC):
                    nc.tensor.matmul(
                        ps[:, :NTILE],
                        lhsT=wf_sb[:, kc, :],
                        rhs=xt_sb[:, kc, nsl],
                        start=(kc == 0),
                        stop=(kc == KC - 1),
                    )
                nc.vector.tensor_copy(fT_sb[:, nsl], ps[:, :NTILE])

            def g_all():
                ps = ps_pair_pool.tile([P, 2 * NTILE], F32, tag="ps_pair",
                                       name="ps_g")
                for half in range(2):
                    msl = slice(half * NTILE, (half + 1) * NTILE)
                    for kc in range(KC):
                        nc.tensor.matmul(
                            ps[:, msl],
                            lhsT=wg_sb[:, kc, :],
                            rhs=xp_sb[:, kc, msl],
                            start=(kc == 0),
                            stop=(kc == KC - 1),
                        )
                nc.vector.tensor_copy(gT_sb, ps)

            def h_chunk(mc):
                ps = ps_pair_pool.tile([P, 2 * NTILE], F32, tag="ps_pair",
                                       name=f"ps_h{mc}")
                for kc in range(KC):
                    nc.tensor.matmul(
                        ps[:, :E],
                        lhsT=xp_sb[:, kc, mc * P : (mc + 1) * P],
                        rhs=wh_sb[:, kc, :],
                        start=(kc == 0),
                        stop=(kc == KC - 1),
                    )
                nc.vector.tensor_copy(h_sb[:, mc, :], ps[:, :E])

            # ---- attention tile machinery ----
            class TileState:
                pass

            def attn_begin(nt):
                st = TileState()
                st.nt = nt
                st.ps_y0 = ps_y0_pool.tile([P, NTILE], F32, tag="ps_y0",
                                           name=f"ps_y0_{nt}")
                st.ps_y1 = ps_y1_pool.tile([P, NTILE], F32, tag="ps_y1",
                                           name=f"ps_y1_{nt}")
                st.ps_sum = ps_sum_pool.tile([P, NTILE], F32, tag="ps_sum",
                                             name=f"ps_sum_{nt}")
                st.ets = {}
                st.ets8 = {}
                return st

            def attn_scores(st, pc):
                # two K=64 score matmuls packed into disjoint PE row groups
                # (fT/gT rows 64:128 hold duplicates), one wide exp
                nt = st.nt
                nsl = slice(nt * NTILE, (nt + 1) * NTILE)
                mcA, mcB = 2 * pc, 2 * pc + 1
                ps_s2 = ps_pair_pool.tile([P, 2 * NTILE], F32, tag="ps_pair",
                                          name=f"ps_s2_{nt}_{pc}")
                nc.tensor.matmul(
                    ps_s2[:, :NTILE],
                    lhsT=gT_sb[0:D, mcA * P : (mcA + 1) * P],
                    rhs=fT_sb[0:D, nsl],
                    start=True, stop=True,
                )
                nc.tensor.matmul(
                    ps_s2[:, NTILE:],
                    lhsT=gT_sb[D : 2 * D, mcB * P : (mcB + 1) * P],
                    rhs=fT_sb[D : 2 * D, nsl],
                    start=True, stop=True,
                )
                et2 = exp_pool.tile([P, 2 * NTILE], F16, tag="et",
                                    name=f"et2_{nt}_{pc}")
                nc.scalar.activation(et2, ps_s2, mybir.ActivationFunctionType.Exp)
                # fp8 copy (DMA-side cast) feeds the DoubleRow row-sum matmul
                et8 = exp8_pool.tile([P, 2, NTILE], F8, tag="et8",
                                     name=f"et8_{nt}_{pc}")
                nc.gpsimd.dma_start(
                    et8, et2[:, :].rearrange("p (i n) -> p i n", i=2)
                )
                st.ets[pc] = et2
                st.ets8[pc] = et8

            def attn_consume(st, pc):
                first = pc == 0
                last = pc == NP - 1
                et2 = st.ets.pop(pc)
                for k in range(2):
                    mc = 2 * pc + k
                    et = et2[:, k * NTILE : (k + 1) * NTILE]
                    nc.tensor.matmul(
                        st.ps_y0, lhsT=h_sb[:, mc, 0:P], rhs=et,
                        start=first and k == 0, stop=last and k == 1,
                    )
                    nc.tensor.matmul(
                        st.ps_y1, lhsT=h_sb[:, mc, P:E], rhs=et,
                        start=first and k == 0, stop=last and k == 1,
                    )
                if pc > 0:
                    sum_dr(st, pc - 1)
                if last:
                    sum_dr(st, pc)

            def sum_dr(st, pc):
                # row sums: one DoubleRow fp8 matmul covers both 128-key
                # chunks of the pair (2x the fp16 rate)
                et8 = st.ets8.pop(pc)
                nc.tensor.matmul(
                    st.ps_sum, lhsT=ones_sb, rhs=et8,
                    perf_mode=mybir.MatmulPerfMode.DoubleRow,
                    start=(pc == 0), stop=(pc == NP - 1),
                )

            def attn_end(st):
                # reciprocal straight off psum (frees ps_sum), then one tiny
                # SBUF->SBUF DMA puts 1/rowsum into per-partition layout for
                # the out-projection drain
                sum_row = small_pool.tile([1, NTILE], F32, tag="sum_row")
                nc.vector.tensor_copy(sum_row, st.ps_sum[0:1, :])
                recip_pt = small_pool.tile([P, NTILE // P], F32, tag="recip_pt")
                if st.nt == NT - 1:
                    # tail-critical: transpose on the (idle) PE instead of the
                    # ~4 us DRAM round trip
                    ps_tr = ps_pair_pool.tile([P, 2 * NTILE], F32,
                                              tag="ps_pair", name="ps_tr")
                    for j in range(NTILE // P):
                        nc.tensor.matmul(
                            ps_tr[:, j : j + 1],
                            lhsT=sum_row[0:1, j * P : (j + 1) * P],
                            rhs=eye_sb[0:1, 0:1],
                            is_transpose=True,
                            start=True, stop=True,
                        )
                    nc.vector.reciprocal(recip_pt, ps_tr[:, 0 : NTILE // P])
                else:
                    nc.sync.dma_start(scr[st.nt : st.nt + 1, :], sum_row)
                    rsum_pt = small_pool.tile([P, NTILE // P], F32,
                                              tag="rsum_pt")
                    nc.sync.dma_start(
                        rsum_pt, scr[st.nt, :].rearrange("(j p) -> p j", p=P)
                    )
                    nc.vector.reciprocal(recip_pt, rsum_pt)
                y_sb = y_pool.tile([P, EC, NTILE], F16, tag="y_sb")
                nc.vector.tensor_copy(y_sb[:, 0, :], st.ps_y0)
                nc.vector.tensor_copy(y_sb[:, 1, :], st.ps_y1)
                return (y_sb, recip_pt, st.nt)

            out_r = out_d.rearrange("(nt j p) c -> p nt j c", p=P, j=NTILE // P)

            def out_chunk(y_prev, recip_pt, nt_prev, j, o_sb, pool=None,
                          tag=None, drain="alt", store="tile"):
                # 128 output rows: matmul into psum; DVE/Act drain applies the
                # per-partition 1/rowsum and casts to fp16; one store per tile
                ps_o = (pool or ps_work_pool).tile(
                    [P, C], F32, tag=tag or "ps_work",
                    name=f"ps_o_{nt_prev}_{j}")
                for ec in range(EC):
                    nc.tensor.matmul(
                        ps_o,
                        lhsT=y_prev[:, ec, j * P : (j + 1) * P],
                        rhs=wo_sb[:, ec, :],
                        start=(ec == 0),
                        stop=(ec == EC - 1),
                    )
                if drain == "act" or (drain == "alt" and j % 2 == 1):
                    nc.scalar.activation(
                        o_sb[:, j, :], ps_o, mybir.ActivationFunctionType.Copy,
                        scale=recip_pt[:, j : j + 1],
                    )
                else:
                    nc.vector.tensor_scalar_mul(
                        o_sb[:, j, :], ps_o, recip_pt[:, j : j + 1]
                    )
                if store == "chunk":
                    eng = (nc.sync, nc.gpsimd, nc.sync, nc.gpsimd)[j]
                    row0 = nt_prev * NTILE + j * P
                    eng.dma_start(out_d[row0 : row0 + P, :], o_sb[:, j, :])
                elif j == NTILE // P - 1:
                    nc.sync.dma_start(out_r[:, nt_prev, :, :], o_sb)

            # ---- emission schedule ----
            # head: f0, f1, g first (gate tile 0's scores); h chunks and the
            # remaining f tiles woven into tile 0..4's pipeline slots so the
            # PE never sits idle waiting on DMA.
            f_tile(0)
            f_tile(1)
            g_all()

            fills = {  # nt -> list of callables to weave into that tile
                0: [lambda m=m: h_chunk(m) for m in range(MC)],
                1: [lambda: f_tile(2), lambda: f_tile(3)],
                2: [lambda: f_tile(4), lambda: f_tile(5)],
                3: [lambda: f_tile(6), lambda: f_tile(7)],
            }

            prev = None
            for nt in range(NT):
                st = attn_begin(nt)
                weave = fills.get(nt, [])
                wi = 0

                def fill(k=1):
                    nonlocal wi
                    for _ in range(k):
                        if wi < len(weave):
                            weave[wi]()
                            wi += 1

                attn_scores(st, 0)
                fill(2)
                o_sb = None
                if prev is not None:
                    o_sb = o_pool.tile([P, NTILE // P, C], F16, tag="o_sb",
                                       name=f"o_sb_{nt}")
                for pc in range(NP):
                    if pc + 1 < NP:
                        attn_scores(st, pc + 1)
                    attn_consume(st, pc)
                    fill(2)
                    if prev is not None and pc < NTILE // P:
                        out_chunk(*prev, pc, o_sb,
                                  drain="act" if nt == NT - 1 else "alt")
                fill(len(weave))
                nxt = attn_end(st)
                prev = nxt

            # final tile's out projection: ps_y0/ps_y1 banks are free now, so
            # alternate three banks to overlap matmul with the drains
            o_sb = o_pool.tile([P, NTILE // P, C], F16, tag="o_sb")
            pools = [(ps_work_pool, "ps_work"), (ps_y0_pool, "ps_y0"),
                     (ps_y1_pool, "ps_y1"), (ps_work_pool, "ps_work")]
            for j in range(NTILE // P):
                pool, tag = pools[j]
                out_chunk(*prev, j, o_sb, pool=pool, tag=tag, store="chunk")

    nc.finalize()
    return nc


def _get_nc():
    if "nc" not in _CACHE:
        _CACHE["nc"] = _build()
    return _CACHE["nc"]


def kernel(x, wf, wg, wh, wo):
    x = np.asarray(x, dtype=np.float32)
    wf = np.asarray(wf, dtype=np.float32)
    wg = np.asarray(wg, dtype=np.float32)
    wh = np.asarray(wh, dtype=np.float32)
    wo = np.asarray(wo, dtype=np.float32)
    B = x.shape[0]
    assert x.shape == (B, 64, 64, C)

    def relay(w, pdim):
        # [pdim*P, d] -> [P, pdim*d]: w[kc*P + p, j] -> out[p, kc*d + j]
        kc, d = w.shape[0] // P, w.shape[1]
        return np.ascontiguousarray(
            w.reshape(kc, P, d).transpose(1, 0, 2).reshape(P, kc * d)
            .astype(np.float16)
        )

    wfr = relay(np.concatenate([wf, wf], axis=1), P)
    wgr = relay(np.concatenate([wg, wg], axis=1), P)
    whr = relay(wh, P)
    wor = relay(wo, P)

    nc = _get_nc()
    in_maps = []
    for b in range(B):
        xb = x[b]
        xt = np.ascontiguousarray(xb.reshape(N, C).T.astype(np.float16))
        xp = xb.reshape(32, 2, 32, 2, C).mean(axis=(1, 3)).reshape(M, C)
        xpt = np.ascontiguousarray(xp.T.astype(np.float16))
        in_maps.append(
            {"xt": xt, "xp": xpt, "wfr": wfr, "wgr": wgr, "whr": whr,
             "wor": wor, "eye": _EYE}
        )

    res = run_bass_kernel_spmd(nc, in_maps, core_ids=list(range(B)))
    kernel.last_result = res

    out = np.empty((B, 64, 64, C), dtype=np.float32)
    for b in range(B):
        out[b] = res.results[b]["out"].astype(np.float32).reshape(64, 64, C)
    return out


# revision 22
# speedup vs baseline: 1.1350x; 1.1350x over previous
"""Pooled-KV attention block on 8 Trainium2 cores, data-parallel over batch.

Reference computation (per batch element b, with x_b: [64, 64, 512] -> [4096, 512]):
    f  = x_b @ wf                     # [4096, 64]
    xp = avgpool2x2(x_b)              # [1024, 512]
    g  = xp @ wg                      # [1024, 64]
    h  = xp @ wh                      # [1024, 256]
    a  = softmax(f @ g.T, axis=-1)    # [4096, 1024]
    y  = a @ h                        # [4096, 256]
    out = y @ wo                      # [4096, 512]

Kernel strategy (one core per batch element, weights replicated). The PE
is the bottleneck (~176k columns of fp16 matmul at 1 col/cycle), so
everything else is arranged to keep it streaming back-to-back:
  - Host supplies x transposed as fp16 (xT [512, 4096]), the 2x2 average
    pooled map xpT [512, 1024] (pooling is linear - fold it into the
    host), and weights pre-transposed to [partition, kc, free] so every
    DMA is a contiguous >=1KB-per-partition burst (strided weight
    rearranges cost ~4us of queue issue time each).
  - A ~7us burst of throwaway matmuls runs while the first input DMAs
    land: it ramps the tensor engine's DVFS clock (otherwise the first
    tiles run at the mid p-state, ~20% slower) and pre-loads the Exp
    activation table.
  - All compute matmuls fp16; intermediates transposed: fT/gT carry
    duplicated 64-row groups so score matmuls pack two 128-key chunks
    into disjoint PE row groups; h is [m, 256] with keys on partitions;
    y accumulates as [e, n] in two psum banks.
  - Softmax skips max-subtraction (|scores| < ~6, exp(fp16) safe). Row
    sums ride a DoubleRow fp8 ones-matmul (2x the fp16 rate): each fp16
    exp tile is copied to e4m3 by a gpsimd DMA (DMA-side dtype cast -
    free engine time), two pairs behind the y-matmuls to hide the DMA
    latency. Quantizing only the row sums costs ~3e-3 relative error.
  - 1/rowsum is transposed to per-partition layout via a DRAM bounce
    (hidden one tile later) and folded into the output-projection drain
    as a per-partition scalar multiply (DVE/Act alternate), which also
    casts to fp16 for the store. The final tile instead transposes its
    row-sum vector on the idle PE (transpose-mode matmuls) to keep the
    ~4us bounce off the tail.
  - Output stored fp16 (host upcasts), halving write traffic; stores
    coalesced per 512-row tile, final tile chunked per 128 rows.
"""

import sys
import types

import numpy as np

import concourse.mybir as mybir
import concourse.tile as tile
from concourse import bacc
from concourse.bass_utils import run_bass_kernel_spmd

# If BASS_TRACE is set but this image's antenv lacks axon_hooks, bass_utils
# would crash on import; provide a no-op hook module so tracing degrades
# gracefully instead (a real hook installed earlier, e.g. by test.py, wins).
try:
    import antenv.axon_hooks  # noqa: F401
except ImportError:
    import antenv

    _stub = types.ModuleType("antenv.axon_hooks")
    _stub._hook = None
    _stub.set_axon_ntff_profile_hook = lambda h: setattr(_stub, "_hook", h)
    _stub.get_axon_ntff_profile_hook = lambda: _stub._hook
    sys.modules["antenv.axon_hooks"] = _stub
    antenv.axon_hooks = _stub

F32 = mybir.dt.float32
F16 = mybir.dt.float16
F8 = mybir.dt.float8e4

P = 128          # SBUF partitions
C = 512          # channels
KC = C // P      # 4 contraction chunks over channels
N = 4096         # query positions (64*64)
NTILE = 512      # n tile (psum free dim)
NT = N // NTILE  # 8 n tiles
M = 1024         # pooled key positions (32*32)
MC = M // P      # 8 key chunks
NP = MC // 2     # 4 score-chunk pairs per n tile
D = 64           # qk head dim
E = 256          # value dim (C//2)
EC = E // P      # 2 value chunks

_CACHE = {}
_EYE = np.eye(128, dtype=np.float32)


def _build():
    nc = bacc.Bacc(None, target_bir_lowering=False)

    xt_d = nc.dram_tensor("xt", [C, N], F16, kind="ExternalInput")
    xp_d = nc.dram_tensor("xp", [C, M], F16, kind="ExternalInput")
    # weights pre-transposed on host to [partition, kc, free] contiguous
    wf_d = nc.dram_tensor("wfr", [P, KC * P], F16, kind="ExternalInput")
    wg_d = nc.dram_tensor("wgr", [P, KC * P], F16, kind="ExternalInput")
    wh_d = nc.dram_tensor("whr", [P, KC * E], F16, kind="ExternalInput")
    wo_d = nc.dram_tensor("wor", [P, EC * C], F16, kind="ExternalInput")
    eye_d = nc.dram_tensor("eye", [P, P], F32, kind="ExternalInput")
    out_d = nc.dram_tensor("out", [N, C], F16, kind="ExternalOutput")

    with tile.TileContext(nc) as tc:
        with (
            tc.tile_pool(name="const", bufs=1) as const_pool,
            tc.tile_pool(name="exp", bufs=4) as exp_pool,
            tc.tile_pool(name="exp8", bufs=4) as exp8_pool,
            tc.tile_pool(name="ysb", bufs=2) as y_pool,
            tc.tile_pool(name="osb", bufs=2) as o_pool,
            tc.tile_pool(name="small", bufs=2) as small_pool,
            tc.tile_pool(name="ps_pair", bufs=2, space="PSUM") as ps_pair_pool,
            tc.tile_pool(name="ps_y0", bufs=1, space="PSUM") as ps_y0_pool,
            tc.tile_pool(name="ps_y1", bufs=1, space="PSUM") as ps_y1_pool,
            tc.tile_pool(name="ps_sum", bufs=1, space="PSUM") as ps_sum_pool,
            tc.tile_pool(name="ps_work", bufs=1, space="PSUM") as ps_work_pool,
            tc.tile_pool(name="scr", bufs=1, space="DRAM") as scr_pool,
        ):
            xt_sb = const_pool.tile([P, KC, N], F16)
            xp_sb = const_pool.tile([P, KC, M], F16)
            wf_sb = const_pool.tile([P, KC, P], F16)
            wg_sb = const_pool.tile([P, KC, P], F16)
            wh_sb = const_pool.tile([P, KC, E], F16)
            wo_sb = const_pool.tile([P, EC, C], F16)
            ones_sb = const_pool.tile([P, 2, P], F8)
            eye_sb = const_pool.tile([P, P], F32)
            fT_sb = const_pool.tile([P, N], F16)
            gT_sb = const_pool.tile([P, M], F16)
            h_sb = const_pool.tile([P, MC, E], F16)

            scr = scr_pool.tile([NT, NTILE], F32)  # reciprocal transpose bounce

            nc.vector.memset(ones_sb, 1.0)  # fp8 ones for DoubleRow sums

            # ---- input DMA, spread over 4 HWDGE queues ----
            # weights first (tiny), then xp (gates g/h -> whole attention),
            # then xt n-tiles in consumption order.
            # xt loads at quarter granularity with kc-paired DMAs so each
            # partition row is a contiguous 2 KB burst (full ring rate);
            # weights + first quarter + xp spread across all three queues
            # so the attention pipeline unblocks ~15 us in.
            xt_r = xt_d.rearrange("(kc p) n -> p kc n", p=P)
            xp_r = xp_d.rearrange("(kc p) m -> p kc m", p=P)
            NQ = N // 4

            def load_quarter_a(eng, q):
                qsl = slice(q * NQ, (q + 1) * NQ)
                eng.dma_start(xt_sb[:, 0:2, qsl], xt_r[:, 0:2, qsl])

            def load_quarter_b(eng, q):
                qsl = slice(q * NQ, (q + 1) * NQ)
                eng.dma_start(xt_sb[:, 2:4, qsl], xt_r[:, 2:4, qsl])

            load_quarter_a(nc.sync, 0)
            load_quarter_b(nc.scalar, 0)
            nc.gpsimd.dma_start(xp_sb[:, 0:2, :], xp_r[:, 0:2, :])
            nc.gpsimd.dma_start(xp_sb[:, 2:4, :], xp_r[:, 2:4, :])
            nc.scalar.dma_start(
                wf_sb, wf_d.rearrange("p (kc d) -> p kc d", kc=KC)
            )
            nc.scalar.dma_start(
                wg_sb, wg_d.rearrange("p (kc d) -> p kc d", kc=KC)
            )
            nc.scalar.dma_start(
                wh_sb, wh_d.rearrange("p (kc e) -> p kc e", kc=KC)
            )
            nc.scalar.dma_start(
                wo_sb, wo_d.rearrange("p (ec c) -> p ec c", ec=EC)
            )
            load_quarter_a(nc.sync, 1)
            load_quarter_b(nc.scalar, 1)
            load_quarter_a(nc.sync, 2)
            load_quarter_b(nc.sync, 2)
            load_quarter_a(nc.sync, 3)
            load_quarter_b(nc.sync, 3)
            nc.sync.dma_start(eye_sb, eye_d[:, :])

            # PE warm-up: ~5 us of throwaway matmuls while the first input
            # DMAs land, so the tensor-engine clock is ramped before real
            # work; also loads the Exp activation table off the critical path
            warm_sb = const_pool.tile([P, NTILE], F16)
            nc.vector.memset(warm_sb, 0.0)
            ps_warm = ps_work_pool.tile([P, C], F32, tag="ps_work",
                                        name="ps_warm")
            NWARM = 36
            for i in range(NWARM):
                nc.tensor.matmul(
                    ps_warm, lhsT=warm_sb[:, 0:P], rhs=warm_sb,
                    start=(i == 0), stop=(i == NWARM - 1),
                )
            warm_e = small_pool.tile([1, 2], F32, tag="warm_e")
            nc.scalar.activation(warm_e, ps_warm[0:1, 0:2],
                                 mybir.ActivationFunctionType.Exp)

            # ---- projection helpers (PE + DVE drain) ----
            def f_tile(nt):
                ps = ps_pair_pool.tile([P, 2 * NTILE], F32, tag="ps_pair",
                                       name=f"ps_f{nt}")
                nsl = slice(nt * NTILE, (nt + 1) * NTILE)
                for kc in range(KC):
                    nc.tensor.matmul(
                        ps[:, :NTILE],
                        lhsT=wf_sb[:, kc, :],
                        rhs=xt_sb[:, kc, nsl],
                        start=(kc == 0),
                        stop=(kc == KC - 1),
                    )
                nc.vector.tensor_copy(fT_sb[:, nsl], ps[:, :NTILE])

            def g_all():
                ps = ps_pair_pool.tile([P, 2 * NTILE], F32, tag="ps_pair",
                                       name="ps_g")
                for half in range(2):
                    msl = slice(half * NTILE, (half + 1) * NTILE)
                    for kc in range(KC):
                        nc.tensor.matmul(
                            ps[:, msl],
                            lhsT=wg_sb[:, kc, :],
                            rhs=xp_sb[:, kc, msl],
                            start=(kc == 0),
                            stop=(kc == KC - 1),
                        )
                nc.vector.tensor_copy(gT_sb, ps)

            def h_chunk(mc):
                ps = ps_pair_pool.tile([P, 2 * NTILE], F32, tag="ps_pair",
                                       name=f"ps_h{mc}")
                for kc in range(KC):
                    nc.tensor.matmul(
                        ps[:, :E],
                        lhsT=xp_sb[:, kc, mc * P : (mc + 1) * P],
                        rhs=wh_sb[:, kc, :],
                        start=(kc == 0),
                        stop=(kc == KC - 1),
                    )
                nc.vector.tensor_copy(h_sb[:, mc, :], ps[:, :E])

            # ---- attention tile machinery ----
            class TileState:
                pass

            def attn_begin(nt):
                st = TileState()
                st.nt = nt
                st.ps_y0 = ps_y0_pool.tile([P, NTILE], F32, tag="ps_y0",
                                           name=f"ps_y0_{nt}")
                st.ps_y1 = ps_y1_pool.tile([P, NTILE], F32, tag="ps_y1",
                                           name=f"ps_y1_{nt}")
                st.ps_sum = ps_sum_pool.tile([P, NTILE], F32, tag="ps_sum",
                                             name=f"ps_sum_{nt}")
                st.ets = {}
                st.ets8 = {}
                return st

            def attn_scores(st, pc):
                # two K=64 score matmuls packed into disjoint PE row groups
                # (fT/gT rows 64:128 hold duplicates), one wide exp
                nt = st.nt
                nsl = slice(nt * NTILE, (nt + 1) * NTILE)
                mcA, mcB = 2 * pc, 2 * pc + 1
                ps_s2 = ps_pair_pool.tile([P, 2 * NTILE], F32, tag="ps_pair",
                                          name=f"ps_s2_{nt}_{pc}")
                nc.tensor.matmul(
                    ps_s2[:, :NTILE],
                    lhsT=gT_sb[0:D, mcA * P : (mcA + 1) * P],
                    rhs=fT_sb[0:D, nsl],
                    start=True, stop=True,
                )
                nc.tensor.matmul(
                    ps_s2[:, NTILE:],
                    lhsT=gT_sb[D : 2 * D, mcB * P : (mcB + 1) * P],
                    rhs=fT_sb[D : 2 * D, nsl],
                    start=True, stop=True,
                )
                et2 = exp_pool.tile([P, 2 * NTILE], F16, tag="et",
                                    name=f"et2_{nt}_{pc}")
                nc.scalar.activation(et2, ps_s2, mybir.ActivationFunctionType.Exp)
                # fp8 copy (DMA-side cast) feeds the DoubleRow row-sum matmul
                et8 = exp8_pool.tile([P, 2, NTILE], F8, tag="et8",
                                     name=f"et8_{nt}_{pc}")
                nc.gpsimd.dma_start(
                    et8, et2[:, :].rearrange("p (i n) -> p i n", i=2)
                )
                st.ets[pc] = et2
                st.ets8[pc] = et8

            def attn_consume(st, pc):
                first = pc == 0
                last = pc == NP - 1
                et2 = st.ets.pop(pc)
                for k in range(2):
                    mc = 2 * pc + k
                    et = et2[:, k * NTILE : (k + 1) * NTILE]
                    nc.tensor.matmul(
                        st.ps_y0, lhsT=h_sb[:, mc, 0:P], rhs=et,
                        start=first and k == 0, stop=last and k == 1,
                    )
                    nc.tensor.matmul(
                        st.ps_y1, lhsT=h_sb[:, mc, P:E], rhs=et,
                        start=first and k == 0, stop=last and k == 1,
                    )
                if pc > 0:
                    sum_dr(st, pc - 1)
                if last:
                    sum_dr(st, pc)

            def sum_dr(st, pc):
                # row sums: one DoubleRow fp8 matmul covers both 128-key
                # chunks of the pair (2x the fp16 rate)
                et8 = st.ets8.pop(pc)
                nc.tensor.matmul(
                    st.ps_sum, lhsT=ones_sb, rhs=et8,
                    perf_mode=mybir.MatmulPerfMode.DoubleRow,
                    start=(pc == 0), stop=(pc == NP - 1),
                )

            def attn_end(st):
                # reciprocal straight off psum (frees ps_sum), then one tiny
                # SBUF->SBUF DMA puts 1/rowsum into per-partition layout for
                # the out-projection drain
                sum_row = small_pool.tile([1, NTILE], F32, tag="sum_row")
                nc.vector.tensor_copy(sum_row, st.ps_sum[0:1, :])
                recip_pt = small_pool.tile([P, NTILE // P], F32, tag="recip_pt")
                if st.nt == NT - 1:
                    # tail-critical: transpose on the (idle) PE instead of the
                    # ~4 us DRAM round trip
                    ps_tr = ps_pair_pool.tile([P, 2 * NTILE], F32,
                                              tag="ps_pair", name="ps_tr")
                    for j in range(NTILE // P):
                        nc.tensor.matmul(
                            ps_tr[:, j : j + 1],
                            lhsT=sum_row[0:1, j * P : (j + 1) * P],
                            rhs=eye_sb[0:1, 0:1],
                            is_transpose=True,
                            start=True, stop=True,
                        )
                    nc.vector.reciprocal(recip_pt, ps_tr[:, 0 : NTILE // P])
                else:
                    nc.sync.dma_start(scr[st.nt : st.nt + 1, :], sum_row)
                    rsum_pt = small_pool.tile([P, NTILE // P], F32,
                                              tag="rsum_pt")
                    nc.sync.dma_start(
                        rsum_pt, scr[st.nt, :].rearrange("(j p) -> p j", p=P)
                    )
                    nc.vector.reciprocal(recip_pt, rsum_pt)
                y_sb = y_pool.tile([P, EC, NTILE], F16, tag="y_sb")
                nc.vector.tensor_copy(y_sb[:, 0, :], st.ps_y0)
                nc.vector.tensor_copy(y_sb[:, 1, :], st.ps_y1)
                return (y_sb, recip_pt, st.nt)

            out_r = out_d.rearrange("(nt j p) c -> p nt j c", p=P, j=NTILE // P)

            def out_chunk(y_prev, recip_pt, nt_prev, j, o_sb, pool=None,
                          tag=None, drain="alt", store="tile"):
                # 128 output rows: matmul into psum; DVE/Act drain applies the
                # per-partition 1/rowsum and casts to fp16; one store per tile
                ps_o = (pool or ps_work_pool).tile(
                    [P, C], F32, tag=tag or "ps_work",
                    name=f"ps_o_{nt_prev}_{j}")
                for ec in range(EC):
                    nc.tensor.matmul(
                        ps_o,
                        lhsT=y_prev[:, ec, j * P : (j + 1) * P],
                        rhs=wo_sb[:, ec, :],
                        start=(ec == 0),
                        stop=(ec == EC - 1),
                    )
                if drain == "act" or (drain == "alt" and j % 2 == 1):
                    nc.scalar.activation(
                        o_sb[:, j, :], ps_o, mybir.ActivationFunctionType.Copy,
                        scale=recip_pt[:, j : j + 1],
                    )
                else:
                    nc.vector.tensor_scalar_mul(
                        o_sb[:, j, :], ps_o, recip_pt[:, j : j + 1]
                    )
                if store == "chunk":
                    eng = (nc.sync, nc.gpsimd, nc.sync, nc.gpsimd)[j]
                    row0 = nt_prev * NTILE + j * P
                    eng.dma_start(out_d[row0 : row0 + P, :], o_sb[:, j, :])
                elif j == NTILE // P - 1:
                    nc.sync.dma_start(out_r[:, nt_prev, :, :], o_sb)

            # ---- emission schedule ----
            # head: f0, f1, g first (gate tile 0's scores); h chunks and the
            # remaining f tiles woven into tile 0..4's pipeline slots so the
            # PE never sits idle waiting on DMA.
            f_tile(0)
            f_tile(1)
            g_all()

            fills = {  # nt -> list of callables to weave into that tile
                0: [lambda m=m: h_chunk(m) for m in range(MC)],
                1: [lambda: f_tile(2), lambda: f_tile(3)],
                2: [lambda: f_tile(4), lambda: f_tile(5)],
                3: [lambda: f_tile(6), lambda: f_tile(7)],
            }

            prev = None
            for nt in range(NT):
                st = attn_begin(nt)
                weave = fills.get(nt, [])
                wi = 0

                def fill(k=1):
                    nonlocal wi
                    for _ in range(k):
                        if wi < len(weave):
                            weave[wi]()
                            wi += 1

                attn_scores(st, 0)
                fill(2)
                o_sb = None
                if prev is not None:
                    o_sb = o_pool.tile([P, NTILE // P, C], F16, tag="o_sb",
                                       name=f"o_sb_{nt}")
                for pc in range(NP):
                    if pc + 1 < NP:
                        attn_scores(st, pc + 1)
                    attn_consume(st, pc)
                    fill(2)
                    if prev is not None and pc < NTILE // P:
                        out_chunk(*prev, pc, o_sb)
                fill(len(weave))
                nxt = attn_end(st)
                prev = nxt

            # final tile's out projection: ps_y0/ps_y1 banks are free now, so
            # alternate three banks to overlap matmul with the drains
            o_sb = o_pool.tile([P, NTILE // P, C], F16, tag="o_sb")
            pools = [(ps_work_pool, "ps_work"), (ps_y0_pool, "ps_y0"),
                     (ps_y1_pool, "ps_y1"), (ps_work_pool, "ps_work")]
            for j in range(NTILE // P):
                pool, tag = pools[j]
                out_chunk(*prev, j, o_sb, pool=pool, tag=tag, store="chunk")

    nc.finalize()
    return nc


def _get_nc():
    if "nc" not in _CACHE:
        _CACHE["nc"] = _build()
    return _CACHE["nc"]


def kernel(x, wf, wg, wh, wo):
    x = np.asarray(x, dtype=np.float32)
    wf = np.asarray(wf, dtype=np.float32)
    wg = np.asarray(wg, dtype=np.float32)
    wh = np.asarray(wh, dtype=np.float32)
    wo = np.asarray(wo, dtype=np.float32)
    B = x.shape[0]
    assert x.shape == (B, 64, 64, C)

    def relay(w, pdim):
        # [pdim*P, d] -> [P, pdim*d]: w[kc*P + p, j] -> out[p, kc*d + j]
        kc, d = w.shape[0] // P, w.shape[1]
        return np.ascontiguousarray(
            w.reshape(kc, P, d).transpose(1, 0, 2).reshape(P, kc * d)
            .astype(np.float16)
        )

    wfr = relay(np.concatenate([wf, wf], axis=1), P)
    wgr = relay(np.concatenate([wg, wg], axis=1), P)
    whr = relay(wh, P)
    wor = relay(wo, P)

    nc = _get_nc()
    in_maps = []
    for b in range(B):
        xb = x[b]
        xt = np.ascontiguousarray(xb.reshape(N, C).T.astype(np.float16))
        xp = xb.reshape(32, 2, 32, 2, C).mean(axis=(1, 3)).reshape(M, C)
        xpt = np.ascontiguousarray(xp.T.astype(np.float16))
        in_maps.append(
            {"xt": xt, "xp": xpt, "wfr": wfr, "wgr": wgr, "whr": whr,
             "wor": wor, "eye": _EYE}
        )

    res = run_bass_kernel_spmd(nc, in_maps, core_ids=list(range(B)))
    kernel.last_result = res

    out = np.empty((B, 64, 64, C), dtype=np.float32)
    for b in range(B):
        out[b] = res.results[b]["out"].astype(np.float32).reshape(64, 64, C)
    return out
